# revision 10
# baseline (speedup 1.0000x reference)
"""ChildSum TreeLSTM (B=64 trees, N=512 nodes, D=300) on 8 NeuronCores.

Strategy: data-parallel over trees (8 trees/core). Within a core, nodes are
level-scheduled by height ("waves"); nodes are packed wave-major (sorted by
parent position within each wave) into 128-slot chunks, so child-sum
aggregation becomes small dense matmuls against host-built one-hot selection
blocks.  All matmul traffic is bf16 (PSUM accumulation in fp32).

This revision:
  - one 3-bank Z PSUM tile [128, 1536] (iou 0:900, g 1024:1324) so the
    x-side / hidden streams run as 512-col chunks and the i,o sigmoids batch
    into a single activation.
  - partial windows (wl < 128) no longer stream the full weight matrices;
    their gate pre-activations are computed up-front in dense "batch"
    windows over compacted tail nodes and injected into PSUM per window with
    identity-slice matmuls (contraction over batch slots).
  - SBUF-only elementwise ops are split between the vector and gpsimd
    engines to unload the DVE.
"""

import hashlib
import numpy as np
import ml_dtypes

BF16 = ml_dtypes.bfloat16

D = 300
DC = 100          # d-chunk (3 chunks of 100 partitions)
NCORES = 8
P = 128


# ----------------------------------------------------------------- schedule

class _Sched:
    pass


def _build_schedule(parent):
    """parent: [B, N] int array, parent[b,t] in (t, N]; N = sentinel."""
    B, N = parent.shape
    tpc = B // NCORES

    heights = np.zeros((B, N), np.int32)
    for b in range(B):
        h = np.zeros(N + 1, np.int32)
        pb = parent[b]
        for t in range(N):
            ht = h[t] + 1
            p = pb[t]
            if ht > h[p]:
                h[p] = ht
        heights[b] = h[:N]

    Hs = [int(heights[c * tpc:(c + 1) * tpc].max()) + 1 for c in range(NCORES)]
    H = max(Hs)

    sizes = np.zeros((NCORES, H), np.int64)
    for c in range(NCORES):
        cnt = np.bincount(heights[c * tpc:(c + 1) * tpc].ravel(), minlength=H)
        sizes[c] = cnt
    env_real = sizes.max(0)                     # real envelope size per wave
    c_env = ((env_real + P - 1) // P) * P       # 128-padded for ST addressing
    off = np.zeros(H + 1, np.int64)
    off[1:] = np.cumsum(c_env)
    P_total = int(off[H])
    NCH = (P_total + P - 1) // P

    # per-core packing: waves descending so parent positions exist first
    pos_all = np.full((NCORES, tpc, N), -1, np.int64)
    BIG = np.iinfo(np.int64).max
    for c in range(NCORES):
        w = heights[c * tpc:(c + 1) * tpc]
        pb = parent[c * tpc:(c + 1) * tpc]
        pos = pos_all[c]
        for v in range(H - 1, -1, -1):
            bs, ts = np.nonzero(w == v)
            if len(bs) == 0:
                continue
            pp = np.empty(len(bs), np.int64)
            for i in range(len(bs)):
                p = pb[bs[i], ts[i]]
                pp[i] = pos[bs[i], p] if p < N else BIG
            order = np.argsort(pp, kind="stable")
            pos[bs[order], ts[order]] = off[v] + np.arange(len(bs))

    # parent packed position per packed slot (-1 = sentinel parent or padding)
    parr = np.full((NCORES, NCH * P), -1, np.int64)
    for c in range(NCORES):
        pb = parent[c * tpc:(c + 1) * tpc]
        pos = pos_all[c]
        for b in range(tpc):
            for t in range(N):
                p = pb[b, t]
                parr[c, pos[b, t]] = pos[b, p] if p < N else -1

    # windows: one per 128-chunk; wl = envelope-real width (<= 128)
    windows = []  # (v, start, wl)
    for v in range(H):
        s = int(off[v])
        rem = int(env_real[v])
        while rem > 0:
            wl = min(P, rem)
            windows.append((v, s, wl))
            s += P
            rem -= wl

    # ---- inject batching: windows with wl < 128 get their x-side gate
    # pre-activations from dense batch chunks (computed up-front), injected
    # via identity-slice matmuls.  Pack each window's batch range so it never
    # crosses a 128 boundary (one inject segment per window).
    inj = {}      # wi -> (bchunk, lo)
    bc, lo = 0, 0
    for wi, (v, s, wl) in enumerate(windows):
        if wl >= P:
            continue
        if lo + wl > P:
            bc += 1
            lo = 0
        inj[wi] = (bc, lo)
        lo += wl
    NBT = bc + 1 if inj else 0
    inj_slot = {wi: i for i, wi in enumerate(inj)}
    NI = max(1, len(inj))

    # selection blocks per window: ST chunks containing any child (any core)
    blocks_by_window = []  # list of list of (global_block_idx, kc)
    block_defs = []        # (win_idx, kc, s, wl)
    for wi, (v, s, wl) in enumerate(windows):
        blks = []
        if v > 0:
            chunks = set()
            for c in range(NCORES):
                childpos = np.nonzero((parr[c] >= s) & (parr[c] < s + wl))[0]
                chunks.update((childpos // P).tolist())
            for kc in sorted(chunks):
                blks.append((len(block_defs), kc))
                block_defs.append((wi, kc, s, wl))
        blocks_by_window.append(blks)

    sc = _Sched()
    sc.B, sc.N, sc.tpc, sc.H = B, N, tpc, H
    sc.env_real, sc.c_env, sc.off = env_real, c_env, off
    sc.P_total, sc.NCH = P_total, NCH
    sc.pos_all, sc.parr = pos_all, parr
    sc.windows = windows
    sc.inj, sc.NBT = inj, NBT
    sc.inj_slot, sc.NI = inj_slot, NI
    sc.ndir = sum(1 for wi in range(len(windows)) if wi not in inj)
    # direct windows get a slot in the transposed-x image
    sc.dir_slot = {}
    k = 0
    for wi in range(len(windows)):
        if wi not in inj:
            sc.dir_slot[wi] = k
            k += 1
    sc.blocks_by_window = blocks_by_window
    sc.block_defs = block_defs
    sc.MAXBLK = max(1, max((len(b) for b in blocks_by_window), default=1))
    # flat offsets of each window's block run in the packed sel stream
    sc.selw_off = {}
    run = 0
    for wi, blks in enumerate(blocks_by_window):
        sc.selw_off[wi] = run
        run += len(blks)
    sc.NB = max(1, run)
    return sc


def _build_core_inputs(sc, c, embs, parent):
    """Per-core input arrays (weights are shared, added separately)."""
    tpc, N, NCH = sc.tpc, sc.N, sc.NCH
    pos = sc.pos_all[c]
    pa = NCH * P

    # packed node -> (b_local, t)
    node_b = np.full(pa, -1, np.int64)
    node_t = np.full(pa, -1, np.int64)
    bs, ts = np.nonzero(pos >= 0)
    node_b[pos[bs, ts]] = bs
    node_t[pos[bs, ts]] = ts

    emb_c = embs[c * tpc:(c + 1) * tpc]  # [tpc, N, D]
    x_rows = np.zeros((pa, D), np.float32)
    real = node_b >= 0
    x_rows[real] = emb_c[node_b[real], node_t[real]]

    pb = parent[c * tpc:(c + 1) * tpc]
    xp_rows = np.zeros((pa, D), np.float32)
    pvals = np.where(real, pb[np.maximum(node_b, 0), np.maximum(node_t, 0)], N)
    has_par = real & (pvals < N)
    xp_rows[has_par] = emb_c[node_b[has_par], pvals[has_par]]

    def tr_block(xb, xpb, wl):
        # [128, 2, 3, 128] transposed x / xp (bias row 1.0 at partition DC)
        out = np.zeros((P, 2, 3, P), BF16)
        for r in range(3):
            out[:DC, 0, r, :wl] = xb[:, r * DC:(r + 1) * DC].T
            out[:DC, 1, r, :wl] = xpb[:, r * DC:(r + 1) * DC].T
        out[DC, 0, 2, :wl] = 1.0
        out[DC, 1, 2, :wl] = 1.0
        return out

    # node-major x rows, [128, NCH, 300] so one DMA loads them all
    xr = np.zeros((P, NCH, D), BF16)
    # transposed x/xp for direct windows
    xtr = np.zeros((max(1, sc.ndir), P, 2, 3, P), BF16)
    # transposed x/xp for inject batch chunks (compacted tail nodes)
    xbat = np.zeros((max(1, sc.NBT), P, 2, 3, P), BF16)
    xbat_acc = [np.zeros((P, D), np.float32) for _ in range(max(1, sc.NBT))]
    xbat_accp = [np.zeros((P, D), np.float32) for _ in range(max(1, sc.NBT))]

    for wi, (v, s, wl) in enumerate(sc.windows):
        ch = s // P
        xb = x_rows[s:s + wl]
        xpb = xp_rows[s:s + wl]
        xr[s % P:s % P + wl, ch] = xb.astype(BF16)
        if wi in sc.inj:
            bc, lo = sc.inj[wi]
            xbat_acc[bc][lo:lo + wl] = xb
            xbat_accp[bc][lo:lo + wl] = xpb
        else:
            xtr[sc.dir_slot[wi]] = tr_block(xb.astype(BF16), xpb.astype(BF16), wl)
    for bc in range(sc.NBT):
        xbat[bc] = tr_block(xbat_acc[bc].astype(BF16),
                            xbat_accp[bc].astype(BF16), P)

    # selection blocks, packed per window in SBUF image order:
    # window run of nblk blocks stored as [128 rows, nblk, 128 cols]
    sel = np.zeros((sc.NB, P, P), BF16)
    parr_c = sc.parr[c]
    for wi, blks in enumerate(sc.blocks_by_window):
        if not blks:
            continue
        nblk = len(blks)
        v, s, wl = sc.windows[wi]
        arr = np.zeros((P, nblk, P), BF16)
        kc2bi = {kc: bi for bi, (gbi, kc) in enumerate(blks)}
        childpos = np.nonzero((parr_c >= s) & (parr_c < s + wl))[0]
        for p in childpos:
            arr[int(p % P), kc2bi[int(p // P)], parr_c[p] - s] = 1.0
        o = sc.selw_off[wi]
        sel[o:o + nblk] = arr.reshape(nblk, P, P)

    injsel = np.zeros((P, sc.NI, P), BF16)
    for wi, (bc, lo) in sc.inj.items():
        wl = sc.windows[wi][2]
        sl = sc.inj_slot[wi]
        for i in range(wl):
            injsel[lo + i, sl, i] = 1.0

    return {
        "xr": xr,
        "xtr": xtr,
        "xbat": xbat,
        "sel": sel,
        "injsel": injsel,
    }


def _shared_weights(Wx, bx, Wh, bh, Wt, bt):
    def chunked_x(Wmat, bias):
        # Wmat: [300, M] -> [128, 3, M] with bias row in chunk 2 (partition
        # dim padded to 128 so the load spreads across DMA queues)
        M = Wmat.shape[1]
        out = np.zeros((P, 3, M), np.float32)
        for r in range(3):
            out[:DC, r] = Wmat[r * DC:(r + 1) * DC]
        out[DC, 2] = bias
        return out.astype(BF16)

    def chunked_h(Wmat):
        M = Wmat.shape[1]
        out = np.zeros((P, 3, M), np.float32)
        for r in range(3):
            out[:DC, r] = Wmat[r * DC:(r + 1) * DC]
        return out.astype(BF16)

    wx_iou = np.concatenate([Wx[0], Wx[1], Wx[2]], axis=1)  # [300, 900]
    wh_iou = np.concatenate([Wh[0], Wh[1], Wh[2]], axis=1)
    b_iou = np.concatenate([bx[0] + bh[0], bx[1] + bh[1], bx[2] + bh[2]])
    return {
        "wioux": chunked_x(wx_iou, b_iou),
        "wiouh": chunked_h(wh_iou),
        "wfx": chunked_x(Wx[3], bx[3] + bh[3]),
        "wfh": chunked_h(Wh[3]),
        "wtt": chunked_x(Wt, bt),
    }


# -------------------------------------------------------------- bass module

# flat-column layout of the Z psum tile [P, 1536] (3 banks):
ZIOU0 = 0          # iou cols 0:512     (bank 0)
ZIOU1 = 512        # iou cols 512:900   (bank 1)
ZG = 1024          # g cols 1024:1324   (bank 2)
# zxs (pre-activation stash for inject windows) columns:
XIOU = 0           # 0:900
XG = 900           # 900:1200
XF = 1200          # 1200:1500


def _build_bass(sc):
    import concourse.mybir as mybir
    import concourse.tile as tile
    from concourse import bacc
    from concourse.masks import make_identity

    f32 = mybir.dt.float32
    bf16 = mybir.dt.bfloat16
    AF = mybir.ActivationFunctionType
    OP = mybir.AluOpType

    NCH, NB, H = sc.NCH, sc.NB, sc.H
    MAXBLK = sc.MAXBLK
    NBT = max(1, sc.NBT)

    nc = bacc.Bacc()
    xr_d = nc.dram_tensor("xr", [P, NCH, D], bf16, kind="ExternalInput")
    xtr_d = nc.dram_tensor("xtr", [max(1, sc.ndir), P, 2 * 3 * P], bf16,
                           kind="ExternalInput")
    xbat_d = nc.dram_tensor("xbat", [NBT, P, 2 * 3 * P], bf16,
                            kind="ExternalInput")
    sel_d = nc.dram_tensor("sel", [NB, P, P], bf16, kind="ExternalInput")
    injsel_d = nc.dram_tensor("injsel", [P, sc.NI, P], bf16,
                              kind="ExternalInput")
    wioux_d = nc.dram_tensor("wioux", [P, 3, 3 * D], bf16, kind="ExternalInput")
    wiouh_d = nc.dram_tensor("wiouh", [P, 3, 3 * D], bf16, kind="ExternalInput")
    wfx_d = nc.dram_tensor("wfx", [P, 3, D], bf16, kind="ExternalInput")
    wfh_d = nc.dram_tensor("wfh", [P, 3, D], bf16, kind="ExternalInput")
    wtt_d = nc.dram_tensor("wtt", [P, 3, D], bf16, kind="ExternalInput")
    out_d = nc.dram_tensor("out", [NCH, P, D], bf16, kind="ExternalOutput")

    with tile.TileContext(nc) as tc:
        with (
            tc.tile_pool(name="const", bufs=1) as constp,
            tc.tile_pool(name="stp", bufs=1) as stp,
            tc.tile_pool(name="stream", bufs=6) as streamp,
            tc.tile_pool(name="ew", bufs=4) as ewp,
            tc.tile_pool(name="ps", bufs=1, space="PSUM") as psp,
        ):
            ident = constp.tile([P, P], bf16)
            make_identity(nc, ident[:])
            injt = constp.tile([P, sc.NI, P], bf16)
            nc.sync.dma_start(injt[:], injsel_d[:])

            wioux = constp.tile([P, 3, 3 * D], bf16)
            nc.sync.dma_start(wioux[:], wioux_d[:])
            wiouh = constp.tile([P, 3, 3 * D], bf16)
            nc.sync.dma_start(wiouh[:], wiouh_d[:])
            wfx = constp.tile([P, 3, D], bf16)
            nc.sync.dma_start(wfx[:], wfx_d[:])
            wfh = constp.tile([P, 3, D], bf16)
            nc.sync.dma_start(wfh[:], wfh_d[:])
            wtt = constp.tile([P, 3, D], bf16)
            nc.sync.dma_start(wtt[:], wtt_d[:])
            # resident packed state, one tile per 128-slot chunk:
            # [128 slots, 6, 100] = st(300) | fst(300)
            STc = [stp.tile([P, 6, DC], bf16, name=f"stc{ch}", tag=f"stc{ch}")
                   for ch in range(NCH)]
            for ch in range(NCH):
                nc.gpsimd.memset(STc[ch][:], 0.0)
            # pre-activation stash for inject windows (bf16)
            zxs = stp.tile([P, NBT, 1500], bf16, name="zxs", tag="zxs")
            # node-major x rows, resident (one DMA)
            XR = stp.tile([P, NCH, D], bf16, name="xrs", tag="xrs")
            nc.sync.dma_start(XR[:], xr_d[:])

            # PSUM tiles
            zt = psp.tile([P, 3 * 512], f32, tag="zt", name="zt")      # 3 banks
            f_ps = psp.tile([P, 384], f32, tag="f", name="f")          # 1 bank
            fc = psp.tile([P, 384], f32, tag="fc", name="fc")          # 1 bank
            hs = psp.tile([P, 3, P], f32, tag="hs", name="hs")         # 1 bank
            tp = psp.tile([P, 3, P], bf16, tag="tp", name="tp")        # 1 bank

            # ---------------- phase 0: batch windows for inject tail ------
            if sc.NBT:
                for bc in range(sc.NBT):
                    xwb = streamp.tile([P, 2 * 3 * P], bf16, tag="xw")
                    nc.sync.dma_start(xwb[:], xbat_d[bc])
                    # iou: 3k chunks x {0:512, 512:900}
                    for k in range(3):
                        nc.tensor.matmul(
                            zt[:, ZIOU0:ZIOU0 + 512],
                            lhsT=xwb[0:DC + 1, k * P:(k + 1) * P],
                            rhs=wioux[0:DC + 1, k, 0:512],
                            start=(k == 0), stop=(k == 2))
                    for k in range(3):
                        nc.tensor.matmul(
                            zt[:, ZIOU1:900],
                            lhsT=xwb[0:DC + 1, k * P:(k + 1) * P],
                            rhs=wioux[0:DC + 1, k, 512:900],
                            start=(k == 0), stop=(k == 2))
                    for k in range(3):
                        nc.tensor.matmul(
                            zt[:, ZG:ZG + D],
                            lhsT=xwb[0:DC + 1, k * P:(k + 1) * P],
                            rhs=wtt[0:DC + 1, k, :],
                            start=(k == 0), stop=(k == 2))
                    for k in range(3):
                        nc.tensor.matmul(
                            f_ps[:, 0:D],
                            lhsT=xwb[0:DC + 1, (3 + k) * P:(4 + k) * P],
                            rhs=wfx[0:DC + 1, k, :],
                            start=(k == 0), stop=(k == 2))
                    # stash pre-activations (bf16)
                    nc.vector.tensor_copy(zxs[:, bc, XIOU:XIOU + 900],
                                          zt[:, 0:900])
                    nc.scalar.copy(zxs[:, bc, XG:XG + D], zt[:, ZG:ZG + D])
                    nc.scalar.copy(zxs[:, bc, XF:XF + D], f_ps[:, 0:D])

            # ---------------- recurrence over windows ----------------------
            for wi, (v, s, wl) in enumerate(sc.windows):
                ch = s // P
                blks = sc.blocks_by_window[wi]
                nblk = len(blks)
                last_wave = (v == H - 1)
                is_inj = wi in sc.inj

                if not is_inj:
                    xv = streamp.tile([P, 2 * 3 * P], bf16, tag="xw")
                    nc.sync.dma_start(xv[:], xtr_d[sc.dir_slot[wi]])

                hsumT = None
                if v > 0:
                    selt = streamp.tile([P, MAXBLK, P], bf16, tag="sel")
                    o = sc.selw_off[wi]
                    nc.sync.dma_start(selt[:, 0:nblk, :], sel_d[o:o + nblk])
                    # hsumT[f, p] = sum_child st[child, f]
                    for r in range(3):
                        for bi, (gbi, kc) in enumerate(blks):
                            nc.tensor.matmul(
                                hs[0:DC, r, :wl],
                                lhsT=STc[kc][:, r, :],
                                rhs=selt[:, bi, :wl],
                                start=(bi == 0), stop=(bi == nblk - 1))
                    hsumT = ewp.tile([DC, 3, P], bf16, tag="hsumT")
                    nc.vector.tensor_copy(hsumT[:, :, :wl], hs[0:DC, 0:3, :wl])
                    # fc[p, f] = sum_child f*st
                    for bi, (gbi, kc) in enumerate(blks):
                        nc.tensor.matmul(
                            fc[:wl, 0:D],
                            lhsT=selt[:, bi, :wl],
                            rhs=STc[kc][:, 3:6, :],
                            start=(bi == 0), stop=(bi == nblk - 1))

                # ---- Z pre-activations: x-side (direct or inject) ---------
                stop0 = (v == 0)
                if is_inj:
                    bc, lo = sc.inj[wi]
                    sl = sc.inj_slot[wi]
                    nc.tensor.matmul(
                        zt[:wl, ZIOU0:ZIOU0 + 512],
                        lhsT=injt[:, sl, 0:wl],
                        rhs=zxs[:, bc, XIOU:XIOU + 512],
                        start=True, stop=stop0)
                    nc.tensor.matmul(
                        zt[:wl, ZIOU1:900],
                        lhsT=injt[:, sl, 0:wl],
                        rhs=zxs[:, bc, XIOU + 512:XIOU + 900],
                        start=True, stop=stop0)
                    nc.tensor.matmul(
                        zt[:wl, ZG:ZG + D],
                        lhsT=injt[:, sl, 0:wl],
                        rhs=zxs[:, bc, XG:XG + D],
                        start=True, stop=True)
                else:
                    for k in range(3):
                        nc.tensor.matmul(
                            zt[:wl, ZIOU0:ZIOU0 + 512],
                            lhsT=xv[0:DC + 1, k * P:k * P + wl],
                            rhs=wioux[0:DC + 1, k, 0:512],
                            start=(k == 0), stop=(stop0 and k == 2))
                    for k in range(3):
                        nc.tensor.matmul(
                            zt[:wl, ZIOU1:900],
                            lhsT=xv[0:DC + 1, k * P:k * P + wl],
                            rhs=wioux[0:DC + 1, k, 512:900],
                            start=(k == 0), stop=(stop0 and k == 2))
                    for k in range(3):
                        nc.tensor.matmul(
                            zt[:wl, ZG:ZG + D],
                            lhsT=xv[0:DC + 1, k * P:k * P + wl],
                            rhs=wtt[0:DC + 1, k, :],
                            start=(k == 0), stop=(k == 2))

                # ---- hidden side ------------------------------------------
                if v > 0:
                    for k in range(3):
                        nc.tensor.matmul(
                            zt[:wl, ZIOU0:ZIOU0 + 512],
                            lhsT=hsumT[:, k, :wl],
                            rhs=wiouh[0:DC, k, 0:512],
                            start=False, stop=(k == 2))
                    for k in range(3):
                        nc.tensor.matmul(
                            zt[:wl, ZIOU1:900],
                            lhsT=hsumT[:, k, :wl],
                            rhs=wiouh[0:DC, k, 512:900],
                            start=False, stop=(k == 2))

                # ---- activations ------------------------------------------
                io_sb = ewp.tile([P, 2 * D], bf16, tag="io_sb")
                nc.scalar.activation(io_sb[:wl], zt[:wl, 0:2 * D], AF.Sigmoid)
                u_sb = ewp.tile([P, D], bf16, tag="u_sb")
                nc.scalar.activation(u_sb[:wl], zt[:wl, 2 * D:3 * D], AF.Tanh)
                g_sb = ewp.tile([P, D], bf16, tag="g_sb")
                nc.scalar.activation(g_sb[:wl], zt[:wl, ZG:ZG + D], AF.Tanh)

                # ---- elementwise (split DVE / gpsimd) ---------------------
                t_sb = ewp.tile([P, D], bf16, tag="t_sb")
                nc.gpsimd.tensor_tensor(t_sb[:wl], io_sb[:wl, 0:D], u_sb[:wl],
                                        OP.mult)
                c_sb = ewp.tile([P, D], f32, tag="c_sb")
                if v > 0:
                    nc.vector.tensor_tensor(c_sb[:wl], t_sb[:wl],
                                            fc[:wl, 0:D], OP.add)
                else:
                    nc.vector.tensor_copy(c_sb[:wl], t_sb[:wl])
                tc_sb = ewp.tile([P, D], bf16, tag="tc_sb")
                nc.scalar.activation(tc_sb[:wl], c_sb[:wl], AF.Tanh)
                h_sb = ewp.tile([P, D], bf16, tag="h_sb")
                nc.gpsimd.tensor_tensor(h_sb[:wl], io_sb[:wl, D:2 * D],
                                        tc_sb[:wl], OP.mult)
                d_sb = ewp.tile([P, D], bf16, tag="d_sb")
                nc.gpsimd.tensor_tensor(d_sb[:wl], h_sb[:wl], XR[:wl, ch, :],
                                        OP.subtract)
                dg_sb = ewp.tile([P, D], bf16, tag="dg_sb")
                nc.vector.tensor_tensor(dg_sb[:wl], d_sb[:wl], g_sb[:wl],
                                        OP.mult)
                nc.vector.tensor_tensor(STc[ch][:wl, 0:3, :], dg_sb[:wl],
                                        XR[:wl, ch, :], OP.add)
                nc.sync.dma_start(out_d[ch], STc[ch][:, 0:3, :])

                if last_wave:
                    continue

                # stT for the f-gate hidden-side matmul
                for r in range(3):
                    nc.tensor.transpose(tp[0:DC, r, :wl],
                                        STc[ch][:wl, r, :],
                                        ident[:wl, :wl])
                stT = ewp.tile([DC, 3, P], bf16, tag="stT")
                nc.vector.tensor_copy(stT[:, :, :wl], tp[0:DC, 0:3, :wl])

                # f = sigmoid(xp @ Wxf + st @ Whf + b); fst = f * st
                if is_inj:
                    bc, lo = sc.inj[wi]
                    sl = sc.inj_slot[wi]
                    nc.tensor.matmul(
                        f_ps[:wl, 0:D],
                        lhsT=injt[:, sl, 0:wl],
                        rhs=zxs[:, bc, XF:XF + D],
                        start=True, stop=False)
                else:
                    for k in range(3):
                        nc.tensor.matmul(
                            f_ps[:wl, 0:D],
                            lhsT=xv[0:DC + 1, (3 + k) * P:(3 + k) * P + wl],
                            rhs=wfx[0:DC + 1, k, :],
                            start=(k == 0), stop=False)
                for k in range(3):
                    nc.tensor.matmul(
                        f_ps[:wl, 0:D], lhsT=stT[:, k, :wl],
                        rhs=wfh[0:DC, k, :],
                        start=False, stop=(k == 2))
                f_sb = ewp.tile([P, D], bf16, tag="f_sb")
                nc.scalar.activation(f_sb[:wl], f_ps[:wl, 0:D], AF.Sigmoid)
                nc.gpsimd.tensor_tensor(STc[ch][:wl, 3:6, :], f_sb[:wl],
                                        STc[ch][:wl, 0:3, :], OP.mult)

    nc.compile()
    return nc


# ------------------------------------------------------------------- driver

_CACHE = {}
LAST_RESULT = None


def kernel(embs, Wx, bx, Wh, bh, Wt, bt, parent):
    global LAST_RESULT
    embs = np.asarray(embs, np.float32)
    Wx = np.asarray(Wx, np.float32)
    bx = np.asarray(bx, np.float32)
    Wh = np.asarray(Wh, np.float32)
    bh = np.asarray(bh, np.float32)
    Wt = np.asarray(Wt, np.float32)
    bt = np.asarray(bt, np.float32)
    parent = np.asarray(parent, np.int64)

    key = hashlib.sha256(parent.tobytes()).hexdigest()
    if key in _CACHE:
        sc, nc = _CACHE[key]
    else:
        sc = _build_schedule(parent)
        nc = _build_bass(sc)
        _CACHE[key] = (sc, nc)

    wts = _shared_weights(Wx, bx, Wh, bh, Wt, bt)
    in_maps = []
    for c in range(NCORES):
        m = _build_core_inputs(sc, c, embs, parent)
        m["xtr"] = m["xtr"].reshape(m["xtr"].shape[0], P, 2 * 3 * P)
        m["xbat"] = m["xbat"].reshape(m["xbat"].shape[0], P, 2 * 3 * P)
        m.update(wts)
        in_maps.append(m)

    from concourse.bass_utils import run_bass_kernel_spmd
    res = run_bass_kernel_spmd(nc, in_maps, core_ids=list(range(NCORES)))
    LAST_RESULT = res

    B, N = parent.shape
    tpc = B // NCORES
    S = np.zeros((B, N, D), np.float32)
    for c in range(NCORES):
        flat = np.asarray(res.results[c]["out"]).astype(np.float32)
        flat = flat.reshape(sc.NCH * P, D)
        pos = sc.pos_all[c]
        S[c * tpc:(c + 1) * tpc] = flat[pos.reshape(-1)].reshape(tpc, N, D)
    return S


# revision 11
# speedup vs baseline: 1.0898x; 1.0898x over previous
"""ChildSum TreeLSTM (B=64 trees, N=512 nodes, D=300) on 8 NeuronCores.

Strategy: data-parallel over trees (8 trees/core). Within a core, nodes are
level-scheduled by height ("waves"); nodes are packed wave-major (sorted by
parent position within each wave) into 128-slot chunks, so child-sum
aggregation becomes small dense matmuls against host-built one-hot selection
blocks.  All matmul traffic is bf16 (PSUM accumulation in fp32).

This revision:
  - one 3-bank Z PSUM tile [128, 1536] (iou 0:900, g 1024:1324) so the
    x-side / hidden streams run as 512-col chunks and the i,o sigmoids batch
    into a single activation.
  - partial windows (wl < 128) no longer stream the full weight matrices;
    their gate pre-activations are computed up-front in dense "batch"
    windows over compacted tail nodes and injected into PSUM per window with
    identity-slice matmuls (contraction over batch slots).
  - SBUF-only elementwise ops are split between the vector and gpsimd
    engines to unload the DVE.
"""

import hashlib
import numpy as np
import ml_dtypes

BF16 = ml_dtypes.bfloat16

D = 300
DC = 100          # d-chunk (3 chunks of 100 partitions)
NCORES = 8
P = 128


# ----------------------------------------------------------------- schedule

class _Sched:
    pass


def _build_schedule(parent):
    """parent: [B, N] int array, parent[b,t] in (t, N]; N = sentinel."""
    B, N = parent.shape
    tpc = B // NCORES

    heights = np.zeros((B, N), np.int32)
    for b in range(B):
        h = np.zeros(N + 1, np.int32)
        pb = parent[b]
        for t in range(N):
            ht = h[t] + 1
            p = pb[t]
            if ht > h[p]:
                h[p] = ht
        heights[b] = h[:N]

    Hs = [int(heights[c * tpc:(c + 1) * tpc].max()) + 1 for c in range(NCORES)]
    H = max(Hs)

    sizes = np.zeros((NCORES, H), np.int64)
    for c in range(NCORES):
        cnt = np.bincount(heights[c * tpc:(c + 1) * tpc].ravel(), minlength=H)
        sizes[c] = cnt
    env_real = sizes.max(0)                     # real envelope size per wave
    c_env = ((env_real + P - 1) // P) * P       # 128-padded for ST addressing
    off = np.zeros(H + 1, np.int64)
    off[1:] = np.cumsum(c_env)
    P_total = int(off[H])
    NCH = (P_total + P - 1) // P

    # per-core packing: waves descending so parent positions exist first
    pos_all = np.full((NCORES, tpc, N), -1, np.int64)
    BIG = np.iinfo(np.int64).max
    for c in range(NCORES):
        w = heights[c * tpc:(c + 1) * tpc]
        pb = parent[c * tpc:(c + 1) * tpc]
        pos = pos_all[c]
        for v in range(H - 1, -1, -1):
            bs, ts = np.nonzero(w == v)
            if len(bs) == 0:
                continue
            pp = np.empty(len(bs), np.int64)
            for i in range(len(bs)):
                p = pb[bs[i], ts[i]]
                pp[i] = pos[bs[i], p] if p < N else BIG
            order = np.argsort(pp, kind="stable")
            pos[bs[order], ts[order]] = off[v] + np.arange(len(bs))

    # parent packed position per packed slot (-1 = sentinel parent or padding)
    parr = np.full((NCORES, NCH * P), -1, np.int64)
    for c in range(NCORES):
        pb = parent[c * tpc:(c + 1) * tpc]
        pos = pos_all[c]
        for b in range(tpc):
            for t in range(N):
                p = pb[b, t]
                parr[c, pos[b, t]] = pos[b, p] if p < N else -1

    # windows: one per 128-chunk; wl = envelope-real width (<= 128)
    windows = []  # (v, start, wl)
    for v in range(H):
        s = int(off[v])
        rem = int(env_real[v])
        while rem > 0:
            wl = min(P, rem)
            windows.append((v, s, wl))
            s += P
            rem -= wl

    # ---- inject batching: windows with wl < 128 get their x-side gate
    # pre-activations from dense batch chunks (computed up-front), injected
    # via identity-slice matmuls.  Pack each window's batch range so it never
    # crosses a 128 boundary (one inject segment per window).
    inj = {}      # wi -> (bchunk, lo)
    bc, lo = 0, 0
    for wi, (v, s, wl) in enumerate(windows):
        if wl >= P:
            continue
        if lo + wl > P:
            bc += 1
            lo = 0
        inj[wi] = (bc, lo)
        lo += wl
    NBT = bc + 1 if inj else 0
    inj_slot = {wi: i for i, wi in enumerate(inj)}
    NI = max(1, len(inj))

    # selection blocks per window: ST chunks containing any child (any core)
    blocks_by_window = []  # list of list of (global_block_idx, kc)
    block_defs = []        # (win_idx, kc, s, wl)
    for wi, (v, s, wl) in enumerate(windows):
        blks = []
        if v > 0:
            chunks = set()
            for c in range(NCORES):
                childpos = np.nonzero((parr[c] >= s) & (parr[c] < s + wl))[0]
                chunks.update((childpos // P).tolist())
            for kc in sorted(chunks):
                blks.append((len(block_defs), kc))
                block_defs.append((wi, kc, s, wl))
        blocks_by_window.append(blks)

    sc = _Sched()
    sc.B, sc.N, sc.tpc, sc.H = B, N, tpc, H
    sc.env_real, sc.c_env, sc.off = env_real, c_env, off
    sc.P_total, sc.NCH = P_total, NCH
    sc.pos_all, sc.parr = pos_all, parr
    sc.windows = windows
    sc.inj, sc.NBT = inj, NBT
    sc.inj_slot, sc.NI = inj_slot, NI
    sc.ndir = sum(1 for wi in range(len(windows)) if wi not in inj)
    # direct windows get a slot in the transposed-x image
    sc.dir_slot = {}
    k = 0
    for wi in range(len(windows)):
        if wi not in inj:
            sc.dir_slot[wi] = k
            k += 1
    sc.blocks_by_window = blocks_by_window
    sc.block_defs = block_defs
    sc.MAXBLK = max(1, max((len(b) for b in blocks_by_window), default=1))
    # flat offsets of each window's block run in the packed sel stream
    sc.selw_off = {}
    run = 0
    for wi, blks in enumerate(blocks_by_window):
        sc.selw_off[wi] = run
        run += len(blks)
    sc.NB = max(1, run)
    return sc


def _build_core_inputs(sc, c, embs, parent):
    """Per-core input arrays (weights are shared, added separately)."""
    tpc, N, NCH = sc.tpc, sc.N, sc.NCH
    pos = sc.pos_all[c]
    pa = NCH * P

    # packed node -> (b_local, t)
    node_b = np.full(pa, -1, np.int64)
    node_t = np.full(pa, -1, np.int64)
    bs, ts = np.nonzero(pos >= 0)
    node_b[pos[bs, ts]] = bs
    node_t[pos[bs, ts]] = ts

    emb_c = embs[c * tpc:(c + 1) * tpc]  # [tpc, N, D]
    x_rows = np.zeros((pa, D), np.float32)
    real = node_b >= 0
    x_rows[real] = emb_c[node_b[real], node_t[real]]

    pb = parent[c * tpc:(c + 1) * tpc]
    xp_rows = np.zeros((pa, D), np.float32)
    pvals = np.where(real, pb[np.maximum(node_b, 0), np.maximum(node_t, 0)], N)
    has_par = real & (pvals < N)
    xp_rows[has_par] = emb_c[node_b[has_par], pvals[has_par]]

    def tr_block(xb, xpb, wl):
        # [128, 2, 3, 128] transposed x / xp (bias row 1.0 at partition DC)
        out = np.zeros((P, 2, 3, P), BF16)
        for r in range(3):
            out[:DC, 0, r, :wl] = xb[:, r * DC:(r + 1) * DC].T
            out[:DC, 1, r, :wl] = xpb[:, r * DC:(r + 1) * DC].T
        out[DC, 0, 2, :wl] = 1.0
        out[DC, 1, 2, :wl] = 1.0
        return out

    # node-major x rows, [128, NCH, 300] so one DMA loads them all
    xr = np.zeros((P, NCH, D), BF16)
    # transposed x/xp for direct windows
    xtr = np.zeros((max(1, sc.ndir), P, 2, 3, P), BF16)
    # transposed x/xp for inject batch chunks (compacted tail nodes)
    xbat = np.zeros((max(1, sc.NBT), P, 2, 3, P), BF16)
    xbat_acc = [np.zeros((P, D), np.float32) for _ in range(max(1, sc.NBT))]
    xbat_accp = [np.zeros((P, D), np.float32) for _ in range(max(1, sc.NBT))]

    for wi, (v, s, wl) in enumerate(sc.windows):
        ch = s // P
        xb = x_rows[s:s + wl]
        xpb = xp_rows[s:s + wl]
        xr[s % P:s % P + wl, ch] = xb.astype(BF16)
        if wi in sc.inj:
            bc, lo = sc.inj[wi]
            xbat_acc[bc][lo:lo + wl] = xb
            xbat_accp[bc][lo:lo + wl] = xpb
        else:
            xtr[sc.dir_slot[wi]] = tr_block(xb.astype(BF16), xpb.astype(BF16), wl)
    for bc in range(sc.NBT):
        xbat[bc] = tr_block(xbat_acc[bc].astype(BF16),
                            xbat_accp[bc].astype(BF16), P)

    # selection blocks, packed per window in SBUF image order:
    # window run of nblk blocks stored as [128 rows, nblk, 128 cols]
    sel = np.zeros((sc.NB, P, P), BF16)
    parr_c = sc.parr[c]
    for wi, blks in enumerate(sc.blocks_by_window):
        if not blks:
            continue
        nblk = len(blks)
        v, s, wl = sc.windows[wi]
        arr = np.zeros((P, nblk, P), BF16)
        kc2bi = {kc: bi for bi, (gbi, kc) in enumerate(blks)}
        childpos = np.nonzero((parr_c >= s) & (parr_c < s + wl))[0]
        for p in childpos:
            arr[int(p % P), kc2bi[int(p // P)], parr_c[p] - s] = 1.0
        o = sc.selw_off[wi]
        sel[o:o + nblk] = arr.reshape(nblk, P, P)

    injsel = np.zeros((P, sc.NI, P), BF16)
    for wi, (bc, lo) in sc.inj.items():
        wl = sc.windows[wi][2]
        sl = sc.inj_slot[wi]
        for i in range(wl):
            injsel[lo + i, sl, i] = 1.0

    return {
        "xr": xr,
        "xtr": xtr,
        "xbat": xbat,
        "sel": sel,
        "injsel": injsel,
    }


def _shared_weights(Wx, bx, Wh, bh, Wt, bt):
    def chunked_x(Wmat, bias):
        # Wmat: [300, M] -> [128, 3, M] with bias row in chunk 2 (partition
        # dim padded to 128 so the load spreads across DMA queues)
        M = Wmat.shape[1]
        out = np.zeros((P, 3, M), np.float32)
        for r in range(3):
            out[:DC, r] = Wmat[r * DC:(r + 1) * DC]
        out[DC, 2] = bias
        return out.astype(BF16)

    def chunked_h(Wmat):
        M = Wmat.shape[1]
        out = np.zeros((P, 3, M), np.float32)
        for r in range(3):
            out[:DC, r] = Wmat[r * DC:(r + 1) * DC]
        return out.astype(BF16)

    wx_iou = np.concatenate([Wx[0], Wx[1], Wx[2]], axis=1)  # [300, 900]
    wh_iou = np.concatenate([Wh[0], Wh[1], Wh[2]], axis=1)
    b_iou = np.concatenate([bx[0] + bh[0], bx[1] + bh[1], bx[2] + bh[2]])
    return {
        "wioux": chunked_x(wx_iou, b_iou),
        "wiouh": chunked_h(wh_iou),
        "wfx": chunked_x(Wx[3], bx[3] + bh[3]),
        "wfh": chunked_h(Wh[3]),
        "wtt": chunked_x(Wt, bt),
    }


# -------------------------------------------------------------- bass module

# flat-column layout of the Z psum tile [P, 1536] (3 banks):
ZIOU0 = 0          # iou cols 0:512     (bank 0)
ZIOU1 = 512        # iou cols 512:900   (bank 1)
ZG = 1024          # g cols 1024:1324   (bank 2)
# zxs (pre-activation stash for inject windows) columns:
XIOU = 0           # 0:900
XG = 900           # 900:1200
XF = 1200          # 1200:1500


def _build_bass(sc):
    import concourse.mybir as mybir
    import concourse.tile as tile
    from concourse import bacc
    from concourse.masks import make_identity

    f32 = mybir.dt.float32
    bf16 = mybir.dt.bfloat16
    AF = mybir.ActivationFunctionType
    OP = mybir.AluOpType

    NCH, NB, H = sc.NCH, sc.NB, sc.H
    MAXBLK = sc.MAXBLK
    NBT = max(1, sc.NBT)

    nc = bacc.Bacc()
    xr_d = nc.dram_tensor("xr", [P, NCH, D], bf16, kind="ExternalInput")
    xtr_d = nc.dram_tensor("xtr", [max(1, sc.ndir), P, 2 * 3 * P], bf16,
                           kind="ExternalInput")
    xbat_d = nc.dram_tensor("xbat", [NBT, P, 2 * 3 * P], bf16,
                            kind="ExternalInput")
    sel_d = nc.dram_tensor("sel", [NB, P, P], bf16, kind="ExternalInput")
    injsel_d = nc.dram_tensor("injsel", [P, sc.NI, P], bf16,
                              kind="ExternalInput")
    wioux_d = nc.dram_tensor("wioux", [P, 3, 3 * D], bf16, kind="ExternalInput")
    wiouh_d = nc.dram_tensor("wiouh", [P, 3, 3 * D], bf16, kind="ExternalInput")
    wfx_d = nc.dram_tensor("wfx", [P, 3, D], bf16, kind="ExternalInput")
    wfh_d = nc.dram_tensor("wfh", [P, 3, D], bf16, kind="ExternalInput")
    wtt_d = nc.dram_tensor("wtt", [P, 3, D], bf16, kind="ExternalInput")
    out_d = nc.dram_tensor("out", [NCH, P, D], bf16, kind="ExternalOutput")

    with tile.TileContext(nc) as tc:
        with (
            tc.tile_pool(name="const", bufs=1) as constp,
            tc.tile_pool(name="stp", bufs=1) as stp,
            tc.tile_pool(name="stream", bufs=6) as streamp,
            tc.tile_pool(name="ew", bufs=4) as ewp,
            tc.tile_pool(name="ps", bufs=1, space="PSUM") as psp,
        ):
            ident = constp.tile([P, P], bf16)
            make_identity(nc, ident[:])
            injt = constp.tile([P, sc.NI, P], bf16)
            nc.sync.dma_start(injt[:], injsel_d[:])

            wioux = constp.tile([P, 3, 3 * D], bf16)
            nc.sync.dma_start(wioux[:], wioux_d[:])
            wiouh = constp.tile([P, 3, 3 * D], bf16)
            nc.sync.dma_start(wiouh[:], wiouh_d[:])
            wfx = constp.tile([P, 3, D], bf16)
            nc.sync.dma_start(wfx[:], wfx_d[:])
            wfh = constp.tile([P, 3, D], bf16)
            nc.sync.dma_start(wfh[:], wfh_d[:])
            wtt = constp.tile([P, 3, D], bf16)
            nc.sync.dma_start(wtt[:], wtt_d[:])
            # resident packed state, one tile per 128-slot chunk:
            # [128 slots, 6, 100] = st(300) | fst(300)
            STc = [stp.tile([P, 6, DC], bf16, name=f"stc{ch}", tag=f"stc{ch}")
                   for ch in range(NCH)]
            for ch in range(NCH):
                nc.gpsimd.memset(STc[ch][:], 0.0)
            # pre-activation stash for inject windows (bf16)
            zxs = stp.tile([P, NBT, 1500], bf16, name="zxs", tag="zxs")
            # node-major x rows, resident (one DMA)
            XR = stp.tile([P, NCH, D], bf16, name="xrs", tag="xrs")
            nc.sync.dma_start(XR[:], xr_d[:])

            # PSUM tiles
            zt = psp.tile([P, 3 * 512], f32, tag="zt", name="zt")      # 3 banks
            f_ps = psp.tile([P, 384], f32, tag="f", name="f")          # 1 bank
            fc = psp.tile([P, 384], f32, tag="fc", name="fc")          # 1 bank
            hs = psp.tile([P, 3, P], f32, tag="hs", name="hs")         # 1 bank
            tp = psp.tile([P, 3, P], bf16, tag="tp", name="tp")        # 1 bank

            # ---------------- phase 0: batch windows for inject tail ------
            if sc.NBT:
                for bc in range(sc.NBT):
                    xwb = streamp.tile([P, 2 * 3 * P], bf16, tag="xw")
                    nc.sync.dma_start(xwb[:], xbat_d[bc])
                    # iou: 3k chunks x {0:512, 512:900}
                    for k in range(3):
                        nc.tensor.matmul(
                            zt[:, ZIOU0:ZIOU0 + 512],
                            lhsT=xwb[0:DC + 1, k * P:(k + 1) * P],
                            rhs=wioux[0:DC + 1, k, 0:512],
                            start=(k == 0), stop=(k == 2))
                    for k in range(3):
                        nc.tensor.matmul(
                            zt[:, ZIOU1:900],
                            lhsT=xwb[0:DC + 1, k * P:(k + 1) * P],
                            rhs=wioux[0:DC + 1, k, 512:900],
                            start=(k == 0), stop=(k == 2))
                    for k in range(3):
                        nc.tensor.matmul(
                            zt[:, ZG:ZG + D],
                            lhsT=xwb[0:DC + 1, k * P:(k + 1) * P],
                            rhs=wtt[0:DC + 1, k, :],
                            start=(k == 0), stop=(k == 2))
                    for k in range(3):
                        nc.tensor.matmul(
                            f_ps[:, 0:D],
                            lhsT=xwb[0:DC + 1, (3 + k) * P:(4 + k) * P],
                            rhs=wfx[0:DC + 1, k, :],
                            start=(k == 0), stop=(k == 2))
                    # stash pre-activations (bf16)
                    nc.vector.tensor_copy(zxs[:, bc, XIOU:XIOU + 900],
                                          zt[:, 0:900])
                    nc.scalar.copy(zxs[:, bc, XG:XG + D], zt[:, ZG:ZG + D])
                    nc.scalar.copy(zxs[:, bc, XF:XF + D], f_ps[:, 0:D])

            # ---------------- recurrence over windows ----------------------
            for wi, (v, s, wl) in enumerate(sc.windows):
                ch = s // P
                blks = sc.blocks_by_window[wi]
                nblk = len(blks)
                last_wave = (v == H - 1)
                is_inj = wi in sc.inj

                if not is_inj:
                    xv = streamp.tile([P, 2 * 3 * P], bf16, tag="xw")
                    nc.sync.dma_start(xv[:], xtr_d[sc.dir_slot[wi]])

                hsumT = None
                if v > 0:
                    selt = streamp.tile([P, MAXBLK, P], bf16, tag="sel")
                    o = sc.selw_off[wi]
                    nc.sync.dma_start(selt[:, 0:nblk, :], sel_d[o:o + nblk])
                    # hsumT[f, p] = sum_child st[child, f]
                    for r in range(3):
                        for bi, (gbi, kc) in enumerate(blks):
                            nc.tensor.matmul(
                                hs[0:DC, r, :wl],
                                lhsT=STc[kc][:, r, :],
                                rhs=selt[:, bi, :wl],
                                start=(bi == 0), stop=(bi == nblk - 1))
                    hsumT = ewp.tile([DC, 3, P], bf16, tag="hsumT")
                    nc.vector.tensor_copy(hsumT[:, :, :wl], hs[0:DC, 0:3, :wl])
                    # fc[p, f] = sum_child f*st
                    for bi, (gbi, kc) in enumerate(blks):
                        nc.tensor.matmul(
                            fc[:wl, 0:D],
                            lhsT=selt[:, bi, :wl],
                            rhs=STc[kc][:, 3:6, :],
                            start=(bi == 0), stop=(bi == nblk - 1))

                # ---- Z pre-activations: x-side (direct or inject) ---------
                stop0 = (v == 0)
                if is_inj:
                    bc, lo = sc.inj[wi]
                    sl = sc.inj_slot[wi]
                    nc.tensor.matmul(
                        zt[:wl, ZIOU0:ZIOU0 + 512],
                        lhsT=injt[:, sl, 0:wl],
                        rhs=zxs[:, bc, XIOU:XIOU + 512],
                        start=True, stop=stop0)
                    nc.tensor.matmul(
                        zt[:wl, ZIOU1:900],
                        lhsT=injt[:, sl, 0:wl],
                        rhs=zxs[:, bc, XIOU + 512:XIOU + 900],
                        start=True, stop=stop0)
                    nc.tensor.matmul(
                        zt[:wl, ZG:ZG + D],
                        lhsT=injt[:, sl, 0:wl],
                        rhs=zxs[:, bc, XG:XG + D],
                        start=True, stop=True)
                else:
                    for k in range(3):
                        nc.tensor.matmul(
                            zt[:wl, ZIOU0:ZIOU0 + 512],
                            lhsT=xv[0:DC + 1, k * P:k * P + wl],
                            rhs=wioux[0:DC + 1, k, 0:512],
                            start=(k == 0), stop=(stop0 and k == 2))
                    for k in range(3):
                        nc.tensor.matmul(
                            zt[:wl, ZIOU1:900],
                            lhsT=xv[0:DC + 1, k * P:k * P + wl],
                            rhs=wioux[0:DC + 1, k, 512:900],
                            start=(k == 0), stop=(stop0 and k == 2))
                    for k in range(3):
                        nc.tensor.matmul(
                            zt[:wl, ZG:ZG + D],
                            lhsT=xv[0:DC + 1, k * P:k * P + wl],
                            rhs=wtt[0:DC + 1, k, :],
                            start=(k == 0), stop=(k == 2))

                # ---- hidden side ------------------------------------------
                if v > 0:
                    for k in range(3):
                        nc.tensor.matmul(
                            zt[:wl, ZIOU0:ZIOU0 + 512],
                            lhsT=hsumT[:, k, :wl],
                            rhs=wiouh[0:DC, k, 0:512],
                            start=False, stop=(k == 2))
                    for k in range(3):
                        nc.tensor.matmul(
                            zt[:wl, ZIOU1:900],
                            lhsT=hsumT[:, k, :wl],
                            rhs=wiouh[0:DC, k, 512:900],
                            start=False, stop=(k == 2))

                # ---- activations ------------------------------------------
                io_sb = ewp.tile([P, 2 * D], bf16, tag="io_sb")
                nc.scalar.activation(io_sb[:wl], zt[:wl, 0:2 * D], AF.Sigmoid)
                u_sb = ewp.tile([P, D], bf16, tag="u_sb")
                nc.scalar.activation(u_sb[:wl], zt[:wl, 2 * D:3 * D], AF.Tanh)
                g_sb = ewp.tile([P, D], bf16, tag="g_sb")
                nc.scalar.activation(g_sb[:wl], zt[:wl, ZG:ZG + D], AF.Tanh)

                # ---- elementwise (split DVE / gpsimd) ---------------------
                t_sb = ewp.tile([P, D], bf16, tag="t_sb")
                nc.vector.tensor_tensor(t_sb[:wl], io_sb[:wl, 0:D], u_sb[:wl],
                                        OP.mult)
                c_sb = ewp.tile([P, D], f32, tag="c_sb")
                if v > 0:
                    nc.vector.tensor_tensor(c_sb[:wl], t_sb[:wl],
                                            fc[:wl, 0:D], OP.add)
                else:
                    nc.vector.tensor_copy(c_sb[:wl], t_sb[:wl])
                tc_sb = ewp.tile([P, D], bf16, tag="tc_sb")
                nc.scalar.activation(tc_sb[:wl], c_sb[:wl], AF.Tanh)
                h_sb = ewp.tile([P, D], bf16, tag="h_sb")
                nc.vector.tensor_tensor(h_sb[:wl], io_sb[:wl, D:2 * D],
                                        tc_sb[:wl], OP.mult)
                d_sb = ewp.tile([P, D], bf16, tag="d_sb")
                nc.vector.tensor_tensor(d_sb[:wl], h_sb[:wl], XR[:wl, ch, :],
                                        OP.subtract)
                dg_sb = ewp.tile([P, D], bf16, tag="dg_sb")
                nc.vector.tensor_tensor(dg_sb[:wl], d_sb[:wl], g_sb[:wl],
                                        OP.mult)
                nc.vector.tensor_tensor(STc[ch][:wl, 0:3, :], dg_sb[:wl],
                                        XR[:wl, ch, :], OP.add)
                nc.scalar.dma_start(out_d[ch], STc[ch][:, 0:3, :])

                if last_wave:
                    continue

                # stT for the f-gate hidden-side matmul
                for r in range(3):
                    nc.tensor.transpose(tp[0:DC, r, :wl],
                                        STc[ch][:wl, r, :],
                                        ident[:wl, :wl])
                stT = ewp.tile([DC, 3, P], bf16, tag="stT")
                nc.vector.tensor_copy(stT[:, :, :wl], tp[0:DC, 0:3, :wl])

                # f = sigmoid(xp @ Wxf + st @ Whf + b); fst = f * st
                if is_inj:
                    bc, lo = sc.inj[wi]
                    sl = sc.inj_slot[wi]
                    nc.tensor.matmul(
                        f_ps[:wl, 0:D],
                        lhsT=injt[:, sl, 0:wl],
                        rhs=zxs[:, bc, XF:XF + D],
                        start=True, stop=False)
                else:
                    for k in range(3):
                        nc.tensor.matmul(
                            f_ps[:wl, 0:D],
                            lhsT=xv[0:DC + 1, (3 + k) * P:(3 + k) * P + wl],
                            rhs=wfx[0:DC + 1, k, :],
                            start=(k == 0), stop=False)
                for k in range(3):
                    nc.tensor.matmul(
                        f_ps[:wl, 0:D], lhsT=stT[:, k, :wl],
                        rhs=wfh[0:DC, k, :],
                        start=False, stop=(k == 2))
                f_sb = ewp.tile([P, D], bf16, tag="f_sb")
                nc.scalar.activation(f_sb[:wl], f_ps[:wl, 0:D], AF.Sigmoid)
                nc.gpsimd.tensor_tensor(STc[ch][:wl, 3:6, :], f_sb[:wl],
                                        STc[ch][:wl, 0:3, :], OP.mult)

    nc.compile()
    return nc


# ------------------------------------------------------------------- driver

_CACHE = {}
LAST_RESULT = None


def kernel(embs, Wx, bx, Wh, bh, Wt, bt, parent):
    global LAST_RESULT
    embs = np.asarray(embs, np.float32)
    Wx = np.asarray(Wx, np.float32)
    bx = np.asarray(bx, np.float32)
    Wh = np.asarray(Wh, np.float32)
    bh = np.asarray(bh, np.float32)
    Wt = np.asarray(Wt, np.float32)
    bt = np.asarray(bt, np.float32)
    parent = np.asarray(parent, np.int64)

    key = hashlib.sha256(parent.tobytes()).hexdigest()
    if key in _CACHE:
        sc, nc = _CACHE[key]
    else:
        sc = _build_schedule(parent)
        nc = _build_bass(sc)
        _CACHE[key] = (sc, nc)

    wts = _shared_weights(Wx, bx, Wh, bh, Wt, bt)
    in_maps = []
    for c in range(NCORES):
        m = _build_core_inputs(sc, c, embs, parent)
        m["xtr"] = m["xtr"].reshape(m["xtr"].shape[0], P, 2 * 3 * P)
        m["xbat"] = m["xbat"].reshape(m["xbat"].shape[0], P, 2 * 3 * P)
        m.update(wts)
        in_maps.append(m)

    from concourse.bass_utils import run_bass_kernel_spmd
    res = run_bass_kernel_spmd(nc, in_maps, core_ids=list(range(NCORES)))
    LAST_RESULT = res

    B, N = parent.shape
    tpc = B // NCORES
    S = np.zeros((B, N, D), np.float32)
    for c in range(NCORES):
        flat = np.asarray(res.results[c]["out"]).astype(np.float32)
        flat = flat.reshape(sc.NCH * P, D)
        pos = sc.pos_all[c]
        S[c * tpc:(c + 1) * tpc] = flat[pos.reshape(-1)].reshape(tpc, N, D)
    return S


# revision 14
# speedup vs baseline: 1.2013x; 1.1023x over previous
"""ChildSum TreeLSTM (B=64 trees, N=512 nodes, D=300) on 8 NeuronCores.

Strategy: data-parallel over trees (8 trees/core). Within a core, nodes are
level-scheduled by height ("waves"); nodes are packed wave-major (sorted by
parent position within each wave) into 128-slot chunks, so child-sum
aggregation becomes small dense matmuls against host-built one-hot selection
blocks.  All matmul traffic is bf16 (PSUM accumulation in fp32).

This revision:
  - one 3-bank Z PSUM tile [128, 1536] (iou 0:900, g 1024:1324) so the
    x-side / hidden streams run as 512-col chunks and the i,o sigmoids batch
    into a single activation.
  - partial windows (wl < 128) no longer stream the full weight matrices;
    their gate pre-activations are computed up-front in dense "batch"
    windows over compacted tail nodes and injected into PSUM per window with
    identity-slice matmuls (contraction over batch slots).
  - SBUF-only elementwise ops are split between the vector and gpsimd
    engines to unload the DVE.
"""

import hashlib
import numpy as np
import ml_dtypes

BF16 = ml_dtypes.bfloat16

D = 300
DC = 100          # d-chunk (3 chunks of 100 partitions)
NCORES = 8
P = 128


# ----------------------------------------------------------------- schedule

class _Sched:
    pass


def _build_schedule(parent):
    """parent: [B, N] int array, parent[b,t] in (t, N]; N = sentinel."""
    B, N = parent.shape
    tpc = B // NCORES

    heights = np.zeros((B, N), np.int32)
    for b in range(B):
        h = np.zeros(N + 1, np.int32)
        pb = parent[b]
        for t in range(N):
            ht = h[t] + 1
            p = pb[t]
            if ht > h[p]:
                h[p] = ht
        heights[b] = h[:N]

    Hs = [int(heights[c * tpc:(c + 1) * tpc].max()) + 1 for c in range(NCORES)]
    H = max(Hs)

    sizes = np.zeros((NCORES, H), np.int64)
    for c in range(NCORES):
        cnt = np.bincount(heights[c * tpc:(c + 1) * tpc].ravel(), minlength=H)
        sizes[c] = cnt
    env_real = sizes.max(0)                     # real envelope size per wave
    c_env = ((env_real + P - 1) // P) * P       # 128-padded for ST addressing
    off = np.zeros(H + 1, np.int64)
    off[1:] = np.cumsum(c_env)
    P_total = int(off[H])
    NCH = (P_total + P - 1) // P

    # per-core packing: waves descending so parent positions exist first
    pos_all = np.full((NCORES, tpc, N), -1, np.int64)
    BIG = np.iinfo(np.int64).max
    for c in range(NCORES):
        w = heights[c * tpc:(c + 1) * tpc]
        pb = parent[c * tpc:(c + 1) * tpc]
        pos = pos_all[c]
        for v in range(H - 1, -1, -1):
            bs, ts = np.nonzero(w == v)
            if len(bs) == 0:
                continue
            pp = np.empty(len(bs), np.int64)
            for i in range(len(bs)):
                p = pb[bs[i], ts[i]]
                pp[i] = pos[bs[i], p] if p < N else BIG
            order = np.argsort(pp, kind="stable")
            pos[bs[order], ts[order]] = off[v] + np.arange(len(bs))

    # parent packed position per packed slot (-1 = sentinel parent or padding)
    parr = np.full((NCORES, NCH * P), -1, np.int64)
    for c in range(NCORES):
        pb = parent[c * tpc:(c + 1) * tpc]
        pos = pos_all[c]
        for b in range(tpc):
            for t in range(N):
                p = pb[b, t]
                parr[c, pos[b, t]] = pos[b, p] if p < N else -1

    # windows: one per 128-chunk; wl = envelope-real width (<= 128)
    windows = []  # (v, start, wl)
    for v in range(H):
        s = int(off[v])
        rem = int(env_real[v])
        while rem > 0:
            wl = min(P, rem)
            windows.append((v, s, wl))
            s += P
            rem -= wl

    # ---- inject batching: windows with wl < 128 get their x-side gate
    # pre-activations from dense batch chunks (computed up-front), injected
    # via identity-slice matmuls.  Pack each window's batch range so it never
    # crosses a 128 boundary (one inject segment per window).
    inj = {}      # wi -> (bchunk, lo)
    bc, lo = 0, 0
    for wi, (v, s, wl) in enumerate(windows):
        if wl >= P:
            continue
        if lo + wl > P:
            bc += 1
            lo = 0
        inj[wi] = (bc, lo)
        lo += wl
    NBT = bc + 1 if inj else 0
    inj_slot = {wi: i for i, wi in enumerate(inj)}
    NI = max(1, len(inj))

    # selection blocks per window: ST chunks containing any child (any core)
    blocks_by_window = []  # list of list of (global_block_idx, kc)
    block_defs = []        # (win_idx, kc, s, wl)
    for wi, (v, s, wl) in enumerate(windows):
        blks = []
        if v > 0:
            chunks = set()
            for c in range(NCORES):
                childpos = np.nonzero((parr[c] >= s) & (parr[c] < s + wl))[0]
                chunks.update((childpos // P).tolist())
            for kc in sorted(chunks):
                blks.append((len(block_defs), kc))
                block_defs.append((wi, kc, s, wl))
        blocks_by_window.append(blks)

    sc = _Sched()
    sc.B, sc.N, sc.tpc, sc.H = B, N, tpc, H
    sc.env_real, sc.c_env, sc.off = env_real, c_env, off
    sc.P_total, sc.NCH = P_total, NCH
    sc.pos_all, sc.parr = pos_all, parr
    sc.windows = windows
    sc.inj, sc.NBT = inj, NBT
    sc.inj_slot, sc.NI = inj_slot, NI
    sc.ndir = sum(1 for wi in range(len(windows)) if wi not in inj)
    # direct windows get a slot in the transposed-x image
    sc.dir_slot = {}
    k = 0
    for wi in range(len(windows)):
        if wi not in inj:
            sc.dir_slot[wi] = k
            k += 1
    sc.blocks_by_window = blocks_by_window
    sc.block_defs = block_defs
    sc.MAXBLK = max(1, max((len(b) for b in blocks_by_window), default=1))
    # flat offsets of each window's block run in the packed sel stream
    sc.selw_off = {}
    run = 0
    for wi, blks in enumerate(blocks_by_window):
        sc.selw_off[wi] = run
        run += len(blks)
    sc.NB = max(1, run)
    return sc


def _build_core_inputs(sc, c, embs, parent):
    """Per-core input arrays (weights are shared, added separately)."""
    tpc, N, NCH = sc.tpc, sc.N, sc.NCH
    pos = sc.pos_all[c]
    pa = NCH * P

    # packed node -> (b_local, t)
    node_b = np.full(pa, -1, np.int64)
    node_t = np.full(pa, -1, np.int64)
    bs, ts = np.nonzero(pos >= 0)
    node_b[pos[bs, ts]] = bs
    node_t[pos[bs, ts]] = ts

    emb_c = embs[c * tpc:(c + 1) * tpc]  # [tpc, N, D]
    x_rows = np.zeros((pa, D), np.float32)
    real = node_b >= 0
    x_rows[real] = emb_c[node_b[real], node_t[real]]

    pb = parent[c * tpc:(c + 1) * tpc]
    xp_rows = np.zeros((pa, D), np.float32)
    pvals = np.where(real, pb[np.maximum(node_b, 0), np.maximum(node_t, 0)], N)
    has_par = real & (pvals < N)
    xp_rows[has_par] = emb_c[node_b[has_par], pvals[has_par]]

    def tr_block(xb, xpb, wl):
        # [128, 2, 3, 128] transposed x / xp (bias row 1.0 at partition DC)
        out = np.zeros((P, 2, 3, P), BF16)
        for r in range(3):
            out[:DC, 0, r, :wl] = xb[:, r * DC:(r + 1) * DC].T
            out[:DC, 1, r, :wl] = xpb[:, r * DC:(r + 1) * DC].T
        out[DC, 0, 2, :wl] = 1.0
        out[DC, 1, 2, :wl] = 1.0
        return out

    # node-major x rows, [128, NCH, 300] so one DMA loads them all
    xr = np.zeros((P, NCH, D), BF16)
    # transposed x/xp for direct windows
    xtr = np.zeros((max(1, sc.ndir), P, 2, 3, P), BF16)
    # transposed x/xp for inject batch chunks (compacted tail nodes)
    xbat = np.zeros((max(1, sc.NBT), P, 2, 3, P), BF16)
    xbat_acc = [np.zeros((P, D), np.float32) for _ in range(max(1, sc.NBT))]
    xbat_accp = [np.zeros((P, D), np.float32) for _ in range(max(1, sc.NBT))]

    for wi, (v, s, wl) in enumerate(sc.windows):
        ch = s // P
        xb = x_rows[s:s + wl]
        xpb = xp_rows[s:s + wl]
        xr[s % P:s % P + wl, ch] = xb.astype(BF16)
        if wi in sc.inj:
            bc, lo = sc.inj[wi]
            xbat_acc[bc][lo:lo + wl] = xb
            xbat_accp[bc][lo:lo + wl] = xpb
        else:
            xtr[sc.dir_slot[wi]] = tr_block(xb.astype(BF16), xpb.astype(BF16), wl)
    for bc in range(sc.NBT):
        xbat[bc] = tr_block(xbat_acc[bc].astype(BF16),
                            xbat_accp[bc].astype(BF16), P)

    # selection blocks, packed per window in SBUF image order:
    # window run of nblk blocks stored as [128 rows, nblk, 128 cols]
    sel = np.zeros((sc.NB, P, P), BF16)
    parr_c = sc.parr[c]
    for wi, blks in enumerate(sc.blocks_by_window):
        if not blks:
            continue
        nblk = len(blks)
        v, s, wl = sc.windows[wi]
        arr = np.zeros((P, nblk, P), BF16)
        kc2bi = {kc: bi for bi, (gbi, kc) in enumerate(blks)}
        childpos = np.nonzero((parr_c >= s) & (parr_c < s + wl))[0]
        for p in childpos:
            arr[int(p % P), kc2bi[int(p // P)], parr_c[p] - s] = 1.0
        o = sc.selw_off[wi]
        sel[o:o + nblk] = arr.reshape(nblk, P, P)

    injsel = np.zeros((P, sc.NI, P), BF16)
    for wi, (bc, lo) in sc.inj.items():
        wl = sc.windows[wi][2]
        sl = sc.inj_slot[wi]
        for i in range(wl):
            injsel[lo + i, sl, i] = 1.0

    return {
        "xr": xr,
        "xtr": xtr,
        "xbat": xbat,
        "sel": sel,
        "injsel": injsel,
    }


def _shared_weights(Wx, bx, Wh, bh, Wt, bt):
    def chunked_x(Wmat, bias):
        # Wmat: [300, M] -> [128, 3, M] with bias row in chunk 2 (partition
        # dim padded to 128 so the load spreads across DMA queues)
        M = Wmat.shape[1]
        out = np.zeros((P, 3, M), np.float32)
        for r in range(3):
            out[:DC, r] = Wmat[r * DC:(r + 1) * DC]
        out[DC, 2] = bias
        return out.astype(BF16)

    def chunked_h(Wmat):
        M = Wmat.shape[1]
        out = np.zeros((P, 3, M), np.float32)
        for r in range(3):
            out[:DC, r] = Wmat[r * DC:(r + 1) * DC]
        return out.astype(BF16)

    wx_iou = np.concatenate([Wx[0], Wx[1], Wx[2]], axis=1)  # [300, 900]
    wh_iou = np.concatenate([Wh[0], Wh[1], Wh[2]], axis=1)
    b_iou = np.concatenate([bx[0] + bh[0], bx[1] + bh[1], bx[2] + bh[2]])
    return {
        "wioux": chunked_x(wx_iou, b_iou),
        "wiouh": chunked_h(wh_iou),
        "wfx": chunked_x(Wx[3], bx[3] + bh[3]),
        "wfh": chunked_h(Wh[3]),
        "wtt": chunked_x(Wt, bt),
    }


# -------------------------------------------------------------- bass module

# flat-column layout of the Z psum tile [P, 1536] (3 banks):
ZIOU0 = 0          # iou cols 0:512     (bank 0)
ZIOU1 = 512        # iou cols 512:900   (bank 1)
ZG = 1024          # g cols 1024:1324   (bank 2)
# zxs (pre-activation stash for inject windows) columns:
XIOU = 0           # 0:900
XG = 900           # 900:1200
XF = 1200          # 1200:1500


def _build_bass(sc):
    import concourse.mybir as mybir
    import concourse.tile as tile
    from concourse import bacc
    from concourse.masks import make_identity

    f32 = mybir.dt.float32
    bf16 = mybir.dt.bfloat16
    AF = mybir.ActivationFunctionType
    OP = mybir.AluOpType

    NCH, NB, H = sc.NCH, sc.NB, sc.H
    MAXBLK = sc.MAXBLK
    NBT = max(1, sc.NBT)

    nc = bacc.Bacc()
    xr_d = nc.dram_tensor("xr", [P, NCH, D], bf16, kind="ExternalInput")
    xtr_d = nc.dram_tensor("xtr", [max(1, sc.ndir), P, 2 * 3 * P], bf16,
                           kind="ExternalInput")
    xbat_d = nc.dram_tensor("xbat", [NBT, P, 2 * 3 * P], bf16,
                            kind="ExternalInput")
    sel_d = nc.dram_tensor("sel", [NB, P, P], bf16, kind="ExternalInput")
    injsel_d = nc.dram_tensor("injsel", [P, sc.NI, P], bf16,
                              kind="ExternalInput")
    wioux_d = nc.dram_tensor("wioux", [P, 3, 3 * D], bf16, kind="ExternalInput")
    wiouh_d = nc.dram_tensor("wiouh", [P, 3, 3 * D], bf16, kind="ExternalInput")
    wfx_d = nc.dram_tensor("wfx", [P, 3, D], bf16, kind="ExternalInput")
    wfh_d = nc.dram_tensor("wfh", [P, 3, D], bf16, kind="ExternalInput")
    wtt_d = nc.dram_tensor("wtt", [P, 3, D], bf16, kind="ExternalInput")
    out_d = nc.dram_tensor("out", [NCH, P, D], bf16, kind="ExternalOutput")

    with tile.TileContext(nc) as tc:
        with (
            tc.tile_pool(name="const", bufs=1) as constp,
            tc.tile_pool(name="stp", bufs=1) as stp,
            tc.tile_pool(name="stream", bufs=6) as streamp,
            tc.tile_pool(name="ew", bufs=4) as ewp,
            tc.tile_pool(name="ps", bufs=1, space="PSUM") as psp,
        ):
            ident = constp.tile([P, P], bf16)
            make_identity(nc, ident[:])
            injt = constp.tile([P, sc.NI, P], bf16)
            nc.sync.dma_start(injt[:], injsel_d[:])

            wioux = constp.tile([P, 3, 3 * D], bf16)
            nc.sync.dma_start(wioux[:], wioux_d[:])
            wiouh = constp.tile([P, 3, 3 * D], bf16)
            nc.sync.dma_start(wiouh[:], wiouh_d[:])
            wfx = constp.tile([P, 3, D], bf16)
            nc.sync.dma_start(wfx[:], wfx_d[:])
            wfh = constp.tile([P, 3, D], bf16)
            nc.sync.dma_start(wfh[:], wfh_d[:])
            wtt = constp.tile([P, 3, D], bf16)
            nc.sync.dma_start(wtt[:], wtt_d[:])
            # resident packed state, one tile per 128-slot chunk:
            # [128 slots, 6, 100] = st(300) | fst(300)
            STc = [stp.tile([P, 6, DC], bf16, name=f"stc{ch}", tag=f"stc{ch}")
                   for ch in range(NCH)]
            # zero only rows that are never written (pad tails of partial
            # windows); real rows are produced before any gather reads them.
            nz = 0
            for wi, (v, s, wl) in enumerate(sc.windows):
                if wl < P:
                    eng = nc.vector if nz % 2 == 0 else nc.gpsimd
                    eng.memset(STc[s // P][:, :, :], 0.0)
                    nz += 1
            # pre-activation stash for inject windows (bf16)
            zxs = stp.tile([P, NBT, 1500], bf16, name="zxs", tag="zxs")
            # node-major x rows, resident (one DMA, issued after the batch
            # loads since it is first consumed late in window 0)
            XR = stp.tile([P, NCH, D], bf16, name="xrs", tag="xrs")

            # PSUM tiles
            zt = psp.tile([P, 3 * 512], f32, tag="zt", name="zt")      # 3 banks
            f_ps = psp.tile([P, 384], f32, tag="f", name="f")          # 1 bank
            fc = psp.tile([P, 384], f32, tag="fc", name="fc")          # 1 bank
            hs = psp.tile([P, 3, P], f32, tag="hs", name="hs")         # 1 bank
            tp = psp.tile([P, 3, P], bf16, tag="tp", name="tp")        # 1 bank

            # ---------------- phase 0: batch windows for inject tail ------
            if sc.NBT:
                for bc in range(sc.NBT):
                    xwb = streamp.tile([P, 2 * 3 * P], bf16, tag="xwb")
                    nc.sync.dma_start(xwb[:], xbat_d[bc])
                    # iou: 3k chunks x {0:512, 512:900}
                    for k in range(3):
                        nc.tensor.matmul(
                            zt[:, ZIOU0:ZIOU0 + 512],
                            lhsT=xwb[0:DC + 1, k * P:(k + 1) * P],
                            rhs=wioux[0:DC + 1, k, 0:512],
                            start=(k == 0), stop=(k == 2))
                    for k in range(3):
                        nc.tensor.matmul(
                            zt[:, ZIOU1:900],
                            lhsT=xwb[0:DC + 1, k * P:(k + 1) * P],
                            rhs=wioux[0:DC + 1, k, 512:900],
                            start=(k == 0), stop=(k == 2))
                    for k in range(3):
                        nc.tensor.matmul(
                            zt[:, ZG:ZG + D],
                            lhsT=xwb[0:DC + 1, k * P:(k + 1) * P],
                            rhs=wtt[0:DC + 1, k, :],
                            start=(k == 0), stop=(k == 2))
                    for k in range(3):
                        nc.tensor.matmul(
                            f_ps[:, 0:D],
                            lhsT=xwb[0:DC + 1, (3 + k) * P:(4 + k) * P],
                            rhs=wfx[0:DC + 1, k, :],
                            start=(k == 0), stop=(k == 2))
                    # stash pre-activations (bf16)
                    nc.vector.tensor_copy(zxs[:, bc, XIOU:XIOU + 900],
                                          zt[:, 0:900])
                    nc.scalar.copy(zxs[:, bc, XG:XG + D], zt[:, ZG:ZG + D])
                    nc.scalar.copy(zxs[:, bc, XF:XF + D], f_ps[:, 0:D])

            nc.sync.dma_start(XR[:], xr_d[:])

            # ---------------- recurrence over windows ----------------------
            for wi, (v, s, wl) in enumerate(sc.windows):
                ch = s // P
                blks = sc.blocks_by_window[wi]
                nblk = len(blks)
                last_wave = (v == H - 1)
                is_inj = wi in sc.inj

                if not is_inj:
                    xv = streamp.tile([P, 2 * 3 * P], bf16, tag="xw")
                    nc.sync.dma_start(xv[:], xtr_d[sc.dir_slot[wi]])

                hsumT = None
                if v > 0:
                    selt = streamp.tile([P, MAXBLK, P], bf16, tag="sel")
                    o = sc.selw_off[wi]
                    nc.sync.dma_start(selt[:, 0:nblk, :], sel_d[o:o + nblk])
                    # hsumT[f, p] = sum_child st[child, f]
                    for r in range(3):
                        for bi, (gbi, kc) in enumerate(blks):
                            nc.tensor.matmul(
                                hs[0:DC, r, :wl],
                                lhsT=STc[kc][:, r, :],
                                rhs=selt[:, bi, :wl],
                                start=(bi == 0), stop=(bi == nblk - 1))
                    hsumT = ewp.tile([DC, 3, P], bf16, tag="hsumT")
                    nc.vector.tensor_copy(hsumT[:, :, :wl], hs[0:DC, 0:3, :wl])

                # ---- Z pre-activations: x-side (direct or inject) ---------
                stop0 = (v == 0)
                if is_inj:
                    bc, lo = sc.inj[wi]
                    sl = sc.inj_slot[wi]
                    nc.tensor.matmul(
                        zt[:wl, ZIOU0:ZIOU0 + 512],
                        lhsT=injt[:, sl, 0:wl],
                        rhs=zxs[:, bc, XIOU:XIOU + 512],
                        start=True, stop=stop0)
                    nc.tensor.matmul(
                        zt[:wl, ZIOU1:900],
                        lhsT=injt[:, sl, 0:wl],
                        rhs=zxs[:, bc, XIOU + 512:XIOU + 900],
                        start=True, stop=stop0)
                    nc.tensor.matmul(
                        zt[:wl, ZG:ZG + D],
                        lhsT=injt[:, sl, 0:wl],
                        rhs=zxs[:, bc, XG:XG + D],
                        start=True, stop=True)
                else:
                    for k in range(3):
                        nc.tensor.matmul(
                            zt[:wl, ZIOU0:ZIOU0 + 512],
                            lhsT=xv[0:DC + 1, k * P:k * P + wl],
                            rhs=wioux[0:DC + 1, k, 0:512],
                            start=(k == 0), stop=(stop0 and k == 2))
                    for k in range(3):
                        nc.tensor.matmul(
                            zt[:wl, ZIOU1:900],
                            lhsT=xv[0:DC + 1, k * P:k * P + wl],
                            rhs=wioux[0:DC + 1, k, 512:900],
                            start=(k == 0), stop=(stop0 and k == 2))
                    for k in range(3):
                        nc.tensor.matmul(
                            zt[:wl, ZG:ZG + D],
                            lhsT=xv[0:DC + 1, k * P:k * P + wl],
                            rhs=wtt[0:DC + 1, k, :],
                            start=(k == 0), stop=(k == 2))

                # ---- hidden side ------------------------------------------
                if v > 0:
                    for k in range(3):
                        nc.tensor.matmul(
                            zt[:wl, ZIOU0:ZIOU0 + 512],
                            lhsT=hsumT[:, k, :wl],
                            rhs=wiouh[0:DC, k, 0:512],
                            start=False, stop=(k == 2))
                    for k in range(3):
                        nc.tensor.matmul(
                            zt[:wl, ZIOU1:900],
                            lhsT=hsumT[:, k, :wl],
                            rhs=wiouh[0:DC, k, 512:900],
                            start=False, stop=(k == 2))

                # fc gather last on the PE: it depends on the previous
                # wave's fst (produced late); hs/x/hidden must not stall on it
                if v > 0:
                    for bi, (gbi, kc) in enumerate(blks):
                        nc.tensor.matmul(
                            fc[:wl, 0:D],
                            lhsT=selt[:, bi, :wl],
                            rhs=STc[kc][:, 3:6, :],
                            start=(bi == 0), stop=(bi == nblk - 1))

                # ---- activations ------------------------------------------
                io_sb = ewp.tile([P, 2 * D], bf16, tag="io_sb")
                nc.scalar.activation(io_sb[:wl], zt[:wl, 0:2 * D], AF.Sigmoid)
                u_sb = ewp.tile([P, D], bf16, tag="u_sb")
                nc.scalar.activation(u_sb[:wl], zt[:wl, 2 * D:3 * D], AF.Tanh)
                g_sb = ewp.tile([P, D], bf16, tag="g_sb")
                nc.scalar.activation(g_sb[:wl], zt[:wl, ZG:ZG + D], AF.Tanh)

                # ---- elementwise (split DVE / gpsimd) ---------------------
                t_sb = ewp.tile([P, D], bf16, tag="t_sb")
                nc.vector.tensor_tensor(t_sb[:wl], io_sb[:wl, 0:D], u_sb[:wl],
                                        OP.mult)
                c_sb = ewp.tile([P, D], f32, tag="c_sb")
                if v > 0:
                    nc.vector.tensor_tensor(c_sb[:wl], t_sb[:wl],
                                            fc[:wl, 0:D], OP.add)
                else:
                    nc.vector.tensor_copy(c_sb[:wl], t_sb[:wl])
                tc_sb = ewp.tile([P, D], bf16, tag="tc_sb")
                nc.scalar.activation(tc_sb[:wl], c_sb[:wl], AF.Tanh)
                h_sb = ewp.tile([P, D], bf16, tag="h_sb")
                nc.vector.tensor_tensor(h_sb[:wl], io_sb[:wl, D:2 * D],
                                        tc_sb[:wl], OP.mult)
                d_sb = ewp.tile([P, D], bf16, tag="d_sb")
                nc.vector.tensor_tensor(d_sb[:wl], h_sb[:wl], XR[:wl, ch, :],
                                        OP.subtract)
                dg_sb = ewp.tile([P, D], bf16, tag="dg_sb")
                nc.vector.tensor_tensor(dg_sb[:wl], d_sb[:wl], g_sb[:wl],
                                        OP.mult)
                nc.vector.tensor_tensor(STc[ch][:wl, 0:3, :], dg_sb[:wl],
                                        XR[:wl, ch, :], OP.add)
                nc.scalar.dma_start(out_d[ch], STc[ch][:, 0:3, :])

                if last_wave:
                    continue

                # stT for the f-gate hidden-side matmul
                for r in range(3):
                    nc.tensor.transpose(tp[0:DC, r, :wl],
                                        STc[ch][:wl, r, :],
                                        ident[:wl, :wl])
                stT = ewp.tile([DC, 3, P], bf16, tag="stT")
                nc.vector.tensor_copy(stT[:, :, :wl], tp[0:DC, 0:3, :wl])

                # f = sigmoid(xp @ Wxf + st @ Whf + b); fst = f * st
                if is_inj:
                    bc, lo = sc.inj[wi]
                    sl = sc.inj_slot[wi]
                    nc.tensor.matmul(
                        f_ps[:wl, 0:D],
                        lhsT=injt[:, sl, 0:wl],
                        rhs=zxs[:, bc, XF:XF + D],
                        start=True, stop=False)
                else:
                    for k in range(3):
                        nc.tensor.matmul(
                            f_ps[:wl, 0:D],
                            lhsT=xv[0:DC + 1, (3 + k) * P:(3 + k) * P + wl],
                            rhs=wfx[0:DC + 1, k, :],
                            start=(k == 0), stop=False)
                for k in range(3):
                    nc.tensor.matmul(
                        f_ps[:wl, 0:D], lhsT=stT[:, k, :wl],
                        rhs=wfh[0:DC, k, :],
                        start=False, stop=(k == 2))
                f_sb = ewp.tile([P, D], bf16, tag="f_sb")
                nc.scalar.activation(f_sb[:wl], f_ps[:wl, 0:D], AF.Sigmoid)
                nc.vector.tensor_tensor(STc[ch][:wl, 3:6, :], f_sb[:wl],
                                        STc[ch][:wl, 0:3, :], OP.mult)

    nc.compile()
    return nc


# ------------------------------------------------------------------- driver

_CACHE = {}
LAST_RESULT = None


def kernel(embs, Wx, bx, Wh, bh, Wt, bt, parent):
    global LAST_RESULT
    embs = np.asarray(embs, np.float32)
    Wx = np.asarray(Wx, np.float32)
    bx = np.asarray(bx, np.float32)
    Wh = np.asarray(Wh, np.float32)
    bh = np.asarray(bh, np.float32)
    Wt = np.asarray(Wt, np.float32)
    bt = np.asarray(bt, np.float32)
    parent = np.asarray(parent, np.int64)

    key = hashlib.sha256(parent.tobytes()).hexdigest()
    if key in _CACHE:
        sc, nc = _CACHE[key]
    else:
        sc = _build_schedule(parent)
        nc = _build_bass(sc)
        _CACHE[key] = (sc, nc)

    wts = _shared_weights(Wx, bx, Wh, bh, Wt, bt)
    in_maps = []
    for c in range(NCORES):
        m = _build_core_inputs(sc, c, embs, parent)
        m["xtr"] = m["xtr"].reshape(m["xtr"].shape[0], P, 2 * 3 * P)
        m["xbat"] = m["xbat"].reshape(m["xbat"].shape[0], P, 2 * 3 * P)
        m.update(wts)
        in_maps.append(m)

    from concourse.bass_utils import run_bass_kernel_spmd
    res = run_bass_kernel_spmd(nc, in_maps, core_ids=list(range(NCORES)))
    LAST_RESULT = res

    B, N = parent.shape
    tpc = B // NCORES
    S = np.zeros((B, N, D), np.float32)
    for c in range(NCORES):
        flat = np.asarray(res.results[c]["out"]).astype(np.float32)
        flat = flat.reshape(sc.NCH * P, D)
        pos = sc.pos_all[c]
        S[c * tpc:(c + 1) * tpc] = flat[pos.reshape(-1)].reshape(tpc, N, D)
    return S
